# revision 6
# baseline (speedup 1.0000x reference)
"""Bass/Tile kernel for nn_Attention_81690277970645 on TRN2 (v2).

Sharding: 8 cores = 2 batches x 4 head-groups (4 heads of d=64 each).
Per core (batch bi, head-group hg):
  inputs:  x_b [2048, 1024], wq/wk/wv slices [1024, 256], bq [256],
           wo slice [256, 1024]
  output:  partial out [2048, 1024] (host sums the 4 head-group partials
           per batch and adds bo + bv @ wo)

v2 design notes (vs v1 baseline at ~317us):
  - The kernel is ScalarE(exp)-bound: 128 exp ops of [128,1024] = ~147us
    minimum ACT time. Everything else must overlap with the exp stream.
  - bf16 end-to-end: x is cast to bf16 on DVE, transposed via the DMA
    xbar (dma_start_transpose, 2-byte dtypes only; out[p,c,s]=in[s,c*128+p]),
    weights cast to bf16 (enables FWL on PE), outT kept bf16 for oproj.
  - bk dropped entirely: softmax_j((q+bq)@(k+bk)) == softmax_j((q+bq)@k)
    since the bk terms are constant in j.  bv folded into the host-side
    gather (out += bv @ wo + bo).
  - Pass order: all 4 i-blocks of head-pair 0, then head-pair 1.  QKV
    projections for pair 1 + output projections are emitted as PE
    fillers inside the ACT-bound S/exp/AV passes.
  - Pass 1 streams against the x DMA: per 4-jt chunk, cast+transpose the
    4 x tiles, project that KT chunk, then S/exp/AV those jt.
  - Row-packed S pairs (2 heads, K=64, auto tile_position from base
    partition 0/64) confirmed concurrent on HW (4ns start gaps).
"""
import sys
import numpy as np

if '/opt/trn_rl_repo' not in sys.path:
    sys.path.insert(0, '/opt/trn_rl_repo')

import concourse.mybir as mybir
from concourse import bacc
from concourse.tile import TileContext

F32 = mybir.dt.float32
F32R = mybir.dt.float32r
BF16 = mybir.dt.bfloat16

SEQ = 2048
DIM = 1024
EMB_C = 256          # per-core emb columns (4 heads x 64)
NH = 4               # heads per core
DH = 64
SCALE = DH ** -0.5
P = 128
NSEQT = SEQ // P     # 16 seq tiles
NDIMC = DIM // P     # 8 dim chunks
NEMBC = EMB_C // P   # 2 emb chunks (= head pairs)
IBLK = 512
NIBLK = SEQ // IBLK  # 4 i-blocks
NJT = SEQ // P       # 16 j tiles
ID = mybir.ActivationFunctionType.Identity
EXP = mybir.ActivationFunctionType.Exp


def build_kernel(row_pack=True):
    nc = bacc.Bacc("TRN2", target_bir_lowering=False, debug=False, num_devices=8)

    x = nc.dram_tensor("x", [SEQ, DIM], F32, kind="ExternalInput")
    wq = nc.dram_tensor("wq", [DIM, EMB_C], F32, kind="ExternalInput")
    wk = nc.dram_tensor("wk", [DIM, EMB_C], F32, kind="ExternalInput")
    wv = nc.dram_tensor("wv", [DIM, EMB_C], F32, kind="ExternalInput")
    bq = nc.dram_tensor("bq", [EMB_C], F32, kind="ExternalInput")
    wo = nc.dram_tensor("wo", [EMB_C, DIM], F32, kind="ExternalInput")
    ones_d = nc.dram_tensor("ones64", [P, DH], F32, kind="ExternalInput")
    out = nc.dram_tensor("out", [SEQ, DIM], F32, kind="ExternalOutput")

    with TileContext(nc) as tc:
        with (
            tc.tile_pool(name="const", bufs=1) as const_pool,
            tc.tile_pool(name="w", bufs=1) as w_pool,
            tc.tile_pool(name="big", bufs=1) as big_pool,
            tc.tile_pool(name="stage", bufs=3) as stage_pool,
            tc.tile_pool(name="ps", bufs=1, space="PSUM") as ps,
        ):
            # ---- constants ----
            ones_t = const_pool.tile([P, DH], F32R)
            nc.sync.dma_start(ones_t[:], ones_d[:].bitcast(F32R))

            # ---- weight DMAs + bf16 casts (wk/wq first: prologue needs them) ----
            w_sbs = {}
            for name, wd in (("wk", wk), ("wq", wq)):
                wf = stage_pool.tile([P, NDIMC, EMB_C], F32, tag="wstage",
                                     bufs=2, name=f"wf_{name}")
                nc.sync.dma_start(wf[:], wd.rearrange("(c p) e -> p c e", p=P))
                wb = w_pool.tile([P, NDIMC, EMB_C], BF16, name=f"wb_{name}")
                nc.any.tensor_copy(wb[:], wf[:])
                w_sbs[name] = wb
            bq_sb = w_pool.tile([P, NEMBC], F32)
            nc.sync.dma_start(bq_sb[:], bq.rearrange("(c p) -> p c", p=P))

            # ---- x DMAs for chunk 0 ahead of the rest ----
            xs_tiles = [None] * NSEQT

            def emit_xdma(k):
                for s in range(4 * k, 4 * k + 4):
                    xs = stage_pool.tile([P, DIM], F32, tag="xs", bufs=5,
                                         name=f"xs_{s}")
                    nc.sync.dma_start(xs[:], x[s * P:(s + 1) * P, :])
                    xs_tiles[s] = xs

            emit_xdma(0)
            emit_xdma(1)

            # wv needed from pass-1 jt0 (VP); wo only in the hp1 round
            for name, wd in (("wv", wv),):
                wf = stage_pool.tile([P, NDIMC, EMB_C], F32, tag="wstage",
                                     bufs=2, name=f"wf_{name}")
                nc.sync.dma_start(wf[:], wd.rearrange("(c p) e -> p c e", p=P))
                wb = w_pool.tile([P, NDIMC, EMB_C], BF16, name=f"wb_{name}")
                nc.any.tensor_copy(wb[:], wf[:])
                w_sbs[name] = wb
            emit_xdma(2)
            emit_xdma(3)
            wof = stage_pool.tile([P, NEMBC, DIM], F32, tag="wostage", bufs=1,
                                  name="wf_wo")
            nc.sync.dma_start(wof[:], wo.rearrange("(c p) n -> p c n", p=P))
            wo_sb = w_pool.tile([P, NEMBC, DIM], BF16, name="wb_wo")
            nc.any.tensor_copy(wo_sb[:], wof[:])

            # ---- big persistent tiles ----
            xT = big_pool.tile([P, NDIMC, SEQ], BF16)    # [dim_low, chunk, seq]
            QT = big_pool.tile([P, NEMBC, SEQ], BF16)
            KT = big_pool.tile([P, NEMBC, SEQ], BF16)
            VP = big_pool.tile([P, NSEQT, NH * (DH + 1)], BF16)
            outT = big_pool.tile([P, NEMBC, SEQ], BF16)
            for h in range(NH):
                nc.vector.memset(VP[:, :, h * (DH + 1) + DH], 1.0)

            # ---- emission helpers ----
            def emit_xchunk(k):
                """casts + xbar transposes for x tiles 4k..4k+3 -> xT cols."""
                for s in range(4 * k, 4 * k + 4):
                    xb = stage_pool.tile([P, DIM], BF16, tag="xb", bufs=4,
                                         name=f"xb_{s}")
                    nc.vector.tensor_copy(xb[:], xs_tiles[s][:])
                    xts = stage_pool.tile([P, NDIMC, P], BF16, tag="xts",
                                          bufs=4, name=f"xts_{s}")
                    nc.sync.dma_start_transpose(xts[:], xb[:])
                    nc.vector.tensor_copy(xT[:, :, s * P:(s + 1) * P], xts[:])

            def emit_proj(dst, wname, pair, ib, eng):
                """dst[:, pair, ib*512:...] = w[:, :, pair]^T @ xT (+bq if QT)."""
                wb = w_sbs[wname]
                pq = ps.tile([P, IBLK], F32, tag="po", bufs=2,
                             name=f"pq_{wname}_{pair}_{ib}")
                for c in range(NDIMC):
                    nc.tensor.matmul(
                        pq[:],
                        wb[:, c, pair * P:(pair + 1) * P],
                        xT[:, c, ib * IBLK:(ib + 1) * IBLK],
                        start=(c == 0), stop=(c == NDIMC - 1),
                    )
                d = dst[:, pair, ib * IBLK:(ib + 1) * IBLK]
                is_q = dst is QT
                if eng == 'act':
                    nc.scalar.activation(
                        d, pq[:], ID,
                        bias=bq_sb[:, pair:pair + 1] if is_q else 0.0,
                        scale=1.0)
                else:
                    if is_q:
                        nc.vector.tensor_scalar_add(d, pq[:],
                                                    bq_sb[:, pair:pair + 1])
                    else:
                        nc.vector.tensor_copy(d, pq[:])

            def emit_vp(s):
                pv = ps.tile([P, EMB_C], F32, tag="po", bufs=2, name=f"pv_{s}")
                for c in range(NDIMC):
                    nc.tensor.matmul(
                        pv[:],
                        xT[:, c, s * P:(s + 1) * P],
                        w_sbs["wv"][:, c, :],
                        start=(c == 0), stop=(c == NDIMC - 1),
                    )
                nc.vector.tensor_copy(
                    VP[:, s, :].rearrange("p (h x) -> p h x", h=NH)[:, :, :DH],
                    pv[:].rearrange("p (h d) -> p h d", h=NH),
                )

            def emit_spair(ib, jt, hp):
                i0 = ib * IBLK
                psS = ps.tile([P, 2, IBLK], F32, tag="s0", bufs=2,
                              name=f"ps{hp}_{ib}_{jt}")
                for hh in range(2):
                    lo = hh * DH
                    nc.tensor.matmul(
                        psS[:, hh, :],
                        KT[lo:lo + DH, hp, jt * P:(jt + 1) * P],
                        QT[lo:lo + DH, hp, i0:i0 + IBLK],
                        start=True, stop=True,
                    )
                es = stage_pool.tile([P, 2, IBLK], BF16, tag="es", bufs=4,
                                     name=f"es{hp}_{ib}_{jt}")
                nc.scalar.activation(es[:], psS[:], EXP, bias=0.0, scale=SCALE)
                return es

            def emit_av(pavs, es, jt, hp):
                for hh in range(2):
                    h = hp * 2 + hh
                    nc.tensor.matmul(
                        pavs[hh][:DH + 1, :],
                        VP[:, jt, h * (DH + 1):(h + 1) * (DH + 1)],
                        es[:, hh, :],
                        start=(jt == 0), stop=(jt == NJT - 1),
                    )

            def make_div(h, ib, pavc, den_row):
                i0 = ib * IBLK

                def go():
                    recb_ps = ps.tile([P, IBLK], F32, tag="po", bufs=2,
                                      name=f"recb_{h}_{ib}")
                    nc.tensor.matmul(
                        recb_ps[:DH, :], ones_t[0:1, :], den_row[:],
                        start=True, stop=True,
                    )
                    recb_sb = stage_pool.tile([DH, IBLK], F32, tag="recb",
                                              bufs=2)
                    nc.vector.reciprocal_approx_fast(recb_sb[:], recb_ps[:DH, :])
                    e_c, e_lo = divmod(h * DH, P)
                    nc.vector.tensor_tensor(
                        outT[e_lo:e_lo + DH, e_c, i0:i0 + IBLK],
                        pavc[:], recb_sb[:], mybir.AluOpType.mult,
                    )
                return go

            def oproj_units(ib):
                units = []
                for s in range(ib * (IBLK // P), (ib + 1) * (IBLK // P)):
                    for nb in range(DIM // IBLK):
                        def go(s=s, nb=nb):
                            po = ps.tile([P, IBLK], F32, tag="po", bufs=2,
                                         name=f"po_{s}_{nb}")
                            for e in range(NEMBC):
                                nc.tensor.matmul(
                                    po[:],
                                    outT[:, e, s * P:(s + 1) * P],
                                    wo_sb[:, e, nb * IBLK:(nb + 1) * IBLK],
                                    start=(e == 0), stop=(e == NEMBC - 1),
                                )
                            oc = stage_pool.tile([P, IBLK], F32, tag="oc",
                                                 bufs=2)
                            nc.vector.tensor_copy(oc[:], po[:])
                            nc.sync.dma_start(
                                out[s * P:(s + 1) * P,
                                    nb * IBLK:(nb + 1) * IBLK], oc[:])
                        units.append(go)
                return units

            # ---- prologue: chunk 0 + first projections ----
            emit_xchunk(0)
            emit_proj(KT, "wk", 0, 0, 'act')
            emit_proj(QT, "wq", 0, 0, 'act')

            # fillers for the hp0 round (consumed at jt % 4 == 2)
            fill = []
            for k in range(4):
                fill.append(lambda k=k: emit_proj(KT, "wk", 1, k, 'vector'))
            for k in range(4):
                fill.append(lambda k=k: emit_proj(QT, "wq", 1, k, 'vector'))

            div2 = []
            pending = []
            for hp in range(2):
                for ib in range(NIBLK):
                    first = (hp == 0 and ib == 0)
                    pavs = [
                        ps.tile([P, IBLK], F32, tag="pav", bufs=2,
                                name=f"pav_{hp}_{hh}_{ib}")
                        for hh in range(2)
                    ]
                    prev = None
                    for jt in range(NJT):
                        if first and jt > 0 and jt % 4 == 0:
                            emit_xchunk(jt // 4)
                            emit_proj(KT, "wk", 0, jt // 4, 'vector')
                        es = emit_spair(ib, jt, hp)
                        if div2 and jt < 2:
                            div2.pop(0)()
                        if prev is not None:
                            emit_av(pavs, prev, jt - 1, hp)
                        if first:
                            emit_vp(jt)
                        else:
                            if fill and jt % 4 == 2:
                                fill.pop(0)()
                            if pending and jt % 2 == 1:
                                pending.pop(0)()
                        prev = es
                    emit_av(pavs, prev, NJT - 1, hp)

                    for hh in range(2):
                        h = hp * 2 + hh
                        pavc = stage_pool.tile([DH, IBLK], F32, tag="pavc",
                                               bufs=2, name=f"pavc_{h}_{ib}")
                        nc.vector.tensor_copy(pavc[:], pavs[hh][:DH, :])
                        den_row = stage_pool.tile([1, IBLK], F32R,
                                                  tag="den_row", bufs=2,
                                                  name=f"den_{h}_{ib}")
                        nc.vector.tensor_copy(
                            den_row[:], pavs[hh][DH:DH + 1, :].bitcast(F32R))
                        div2.append(make_div(h, ib, pavc, den_row))

                    if hp == 0 and ib < NIBLK - 1:
                        # next hp0 pass needs its QT block emitted before its S
                        emit_proj(QT, "wq", 0, ib + 1, 'vector')
                    if hp == 0 and ib == NIBLK - 1:
                        # flush any leftover pair-1 projections before round 2
                        while fill:
                            fill.pop(0)()
                    if hp == 1:
                        pending.extend(oproj_units(ib))

            for go in div2:
                go()
            for go in pending:
                go()

    nc.compile()
    return nc


def shard_inputs(inputs):
    """Full inputs dict -> list of 8 per-core input dicts."""
    x = np.ascontiguousarray(inputs["x"], dtype=np.float32)
    maps = []
    for core in range(8):
        bi, hg = divmod(core, 4)
        sl = slice(hg * EMB_C, (hg + 1) * EMB_C)
        maps.append({
            "x": np.ascontiguousarray(x[bi]),
            "wq": np.ascontiguousarray(inputs["wq"][:, sl], np.float32),
            "wk": np.ascontiguousarray(inputs["wk"][:, sl], np.float32),
            "wv": np.ascontiguousarray(inputs["wv"][:, sl], np.float32),
            "bq": np.ascontiguousarray(inputs["bq"][sl], np.float32),
            "wo": np.ascontiguousarray(inputs["wo"][sl, :], np.float32),
            "ones64": np.ones((P, DH), np.float32),
        })
    return maps


def gather_outputs(results, inputs):
    out = np.zeros((2, SEQ, DIM), np.float32)
    for core in range(8):
        bi = core // 4
        out[bi] += results[core]["out"]
    bo = np.asarray(inputs["bo"], np.float32)
    bv = np.asarray(inputs["bv"], np.float32)
    wo = np.asarray(inputs["wo"], np.float32)
    out += bo + bv @ wo
    return out


_NC_CACHE = {}


def _get_nc(row_pack=True):
    if row_pack not in _NC_CACHE:
        _NC_CACHE[row_pack] = build_kernel(row_pack=row_pack)
    return _NC_CACHE[row_pack]


def run_sharded(inputs, trace=False, row_pack=True):
    """Returns (full_output [2,2048,1024] fp32, BassKernelResults)."""
    from concourse import bass_utils
    nc = _get_nc(row_pack)
    maps = shard_inputs(inputs)
    res = bass_utils.run_bass_kernel_spmd(
        nc, maps, core_ids=list(range(8)), trace=trace,
    )
    out = gather_outputs(res.results, inputs)
    return out, res


def kernel(**inputs):
    out, _ = run_sharded(inputs)
    return out


# revision 8
# speedup vs baseline: 1.1876x; 1.1876x over previous
"""Bass/Tile kernel for nn_Attention_81690277970645 on TRN2 (v3).

Sharding: 8 cores = 2 batches x 4 head-groups (4 heads of d=64 each).
Per core (batch bi, head-group hg):
  inputs:  x_b [2048, 1024], wq/wk/wv slices [1024, 256], bq [256],
           wo slice [256, 1024]
  output:  partial out [2048, 1024] (host sums the 4 head-group partials
           per batch and adds bo + bv @ wo)

The kernel is ScalarE(exp)-bound: 128 exp ops of [128,1024] at ~1.34us
each = ~171us of ACT time.  v3 organizes everything else around a
continuous exp stream:
  - one global stream of 128 (hp, ib, jt) slots; S(t+1) is emitted
    before AV(t) so exp never waits at pass boundaries
  - pass 1 (hp0, ib0) streams against the x DMA: per 4-jt chunk the 4
    x tiles are cast (DVE+GpSimd split) + xbar-transposed straight into
    xT, then that KT chunk is projected
  - all other QKV projections and output projections are PE fillers
    inside the exp-bound passes
  - bf16 everywhere; bk dropped (softmax-invariant), bv folded into the
    host-side gather; row-packed S pairs (K=64 x2 concurrent)
"""
import sys
import numpy as np

if '/opt/trn_rl_repo' not in sys.path:
    sys.path.insert(0, '/opt/trn_rl_repo')

import concourse.mybir as mybir
from concourse import bacc
from concourse.tile import TileContext

F32 = mybir.dt.float32
F32R = mybir.dt.float32r
BF16 = mybir.dt.bfloat16

SEQ = 2048
DIM = 1024
EMB_C = 256          # per-core emb columns (4 heads x 64)
NH = 4               # heads per core
DH = 64
SCALE = DH ** -0.5
P = 128
NSEQT = SEQ // P     # 16 seq tiles
NDIMC = DIM // P     # 8 dim chunks
NEMBC = EMB_C // P   # 2 emb chunks (= head pairs)
IBLK = 512
NIBLK = SEQ // IBLK  # 4 i-blocks
NJT = SEQ // P       # 16 j tiles
ID = mybir.ActivationFunctionType.Identity
EXP = mybir.ActivationFunctionType.Exp


def build_kernel(row_pack=True):
    nc = bacc.Bacc("TRN2", target_bir_lowering=False, debug=False, num_devices=8)

    x = nc.dram_tensor("x", [SEQ, DIM], F32, kind="ExternalInput")
    wq = nc.dram_tensor("wq", [DIM, EMB_C], F32, kind="ExternalInput")
    wk = nc.dram_tensor("wk", [DIM, EMB_C], F32, kind="ExternalInput")
    wv = nc.dram_tensor("wv", [DIM, EMB_C], F32, kind="ExternalInput")
    bq = nc.dram_tensor("bq", [EMB_C], F32, kind="ExternalInput")
    wo = nc.dram_tensor("wo", [EMB_C, DIM], F32, kind="ExternalInput")
    ones_d = nc.dram_tensor("ones64", [P, DH], F32, kind="ExternalInput")
    out = nc.dram_tensor("out", [SEQ, DIM], F32, kind="ExternalOutput")

    with TileContext(nc) as tc:
        with (
            tc.tile_pool(name="const", bufs=1) as const_pool,
            tc.tile_pool(name="w", bufs=1) as w_pool,
            tc.tile_pool(name="big", bufs=1) as big_pool,
            tc.tile_pool(name="stage", bufs=3) as stage_pool,
            tc.tile_pool(name="ps", bufs=1, space="PSUM") as ps,
        ):
            # ---- x DMAs (2-tile granularity, 8 descriptors) + weights ----
            xs_pairs = [None] * 8

            def emit_xdma(j):  # tiles 2j, 2j+1
                xs = stage_pool.tile([P, 2, DIM], F32, tag="xs", bufs=4,
                                     name=f"xs_{j}")
                nc.sync.dma_start(
                    xs[:], x.rearrange("(t p) d -> p t d", p=P)[:, 2 * j:2 * j + 2, :])
                xs_pairs[j] = xs

            emit_xdma(0)
            emit_xdma(1)
            ones_t = const_pool.tile([P, DH], F32R)
            nc.sync.dma_start(ones_t[:], ones_d[:].bitcast(F32R))

            w_sbs = {}

            def emit_wdma(name, wd):
                wf = stage_pool.tile([P, NDIMC, EMB_C], F32, tag="wstage",
                                     bufs=2, name=f"wf_{name}")
                nc.sync.dma_start(wf[:], wd.rearrange("(c p) e -> p c e", p=P))
                wb = w_pool.tile([P, NDIMC, EMB_C], BF16, name=f"wb_{name}")
                nc.vector.tensor_copy(wb[:], wf[:])
                w_sbs[name] = wb

            emit_wdma("wk", wk)
            emit_wdma("wq", wq)
            bq_sb = w_pool.tile([P, NEMBC], F32)
            nc.sync.dma_start(bq_sb[:], bq.rearrange("(c p) -> p c", p=P))

            # ---- big persistent tiles ----
            xT = big_pool.tile([P, NDIMC, SEQ], BF16)    # [dim_low, chunk, seq]
            QT = big_pool.tile([P, NEMBC, SEQ], BF16)
            KT = big_pool.tile([P, NEMBC, SEQ], BF16)
            VP = big_pool.tile([P, NSEQT, NH * (DH + 1)], BF16)
            outT = big_pool.tile([P, NEMBC, SEQ], BF16)
            for h in range(NH):
                nc.vector.memset(VP[:, :, h * (DH + 1) + DH], 1.0)

            # ---- x-pipeline helpers ----
            def emit_xform(k, engines):
                """cast + transpose x tiles 4k..4k+3 into xT."""
                for half in range(2):
                    j = 2 * k + half
                    xb = stage_pool.tile([P, 2, DIM], BF16, tag="xb", bufs=4,
                                         name=f"xb_{j}")
                    eng = engines[half]
                    if eng == 'gpsimd':
                        nc.gpsimd.tensor_copy(xb[:], xs_pairs[j][:])
                    else:
                        nc.vector.tensor_copy(xb[:], xs_pairs[j][:])
                    for t in range(2):
                        s = 2 * j + t
                        nc.sync.dma_start_transpose(
                            xT[:, :, s * P:(s + 1) * P], xb[:, t, :])

            def emit_proj(dst, wname, pair, col0, ncols, eng):
                """dst[:, pair, col0:col0+ncols] = w^T @ xT (+bq if QT)."""
                wb = w_sbs[wname]
                pq = ps.tile([P, IBLK], F32, tag="po", bufs=2,
                             name=f"pq_{wname}_{pair}_{col0}")
                for c in range(NDIMC):
                    nc.tensor.matmul(
                        pq[:, :ncols],
                        wb[:, c, pair * P:(pair + 1) * P],
                        xT[:, c, col0:col0 + ncols],
                        start=(c == 0), stop=(c == NDIMC - 1),
                    )
                d = dst[:, pair, col0:col0 + ncols]
                is_q = dst is QT
                if eng == 'act':
                    nc.scalar.activation(
                        d, pq[:, :ncols], ID,
                        bias=bq_sb[:, pair:pair + 1] if is_q else 0.0,
                        scale=1.0)
                else:
                    if is_q:
                        nc.vector.tensor_scalar_add(d, pq[:, :ncols],
                                                    bq_sb[:, pair:pair + 1])
                    else:
                        nc.vector.tensor_copy(d, pq[:, :ncols])

            def emit_vp(s):
                pv = ps.tile([P, EMB_C], F32, tag="po", bufs=2, name=f"pv_{s}")
                for c in range(NDIMC):
                    nc.tensor.matmul(
                        pv[:],
                        xT[:, c, s * P:(s + 1) * P],
                        w_sbs["wv"][:, c, :],
                        start=(c == 0), stop=(c == NDIMC - 1),
                    )
                nc.vector.tensor_copy(
                    VP[:, s, :].rearrange("p (h x) -> p h x", h=NH)[:, :, :DH],
                    pv[:].rearrange("p (h d) -> p h d", h=NH),
                )

            def emit_spair(ib, jt, hp):
                i0 = ib * IBLK
                psS = ps.tile([P, 2, IBLK], F32, tag="s0", bufs=2,
                              name=f"ps{hp}_{ib}_{jt}")
                for hh in range(2):
                    lo = hh * DH
                    nc.tensor.matmul(
                        psS[:, hh, :],
                        KT[lo:lo + DH, hp, jt * P:(jt + 1) * P],
                        QT[lo:lo + DH, hp, i0:i0 + IBLK],
                        start=True, stop=True,
                    )
                es = stage_pool.tile([P, 2, IBLK], BF16, tag="es", bufs=6,
                                     name=f"es{hp}_{ib}_{jt}")
                nc.scalar.activation(es[:], psS[:], EXP, bias=0.0, scale=SCALE)
                return es

            def emit_av(pavs, es, jt, hp):
                for hh in range(2):
                    h = hp * 2 + hh
                    nc.tensor.matmul(
                        pavs[hh][:DH + 1, :],
                        VP[:, jt, h * (DH + 1):(h + 1) * (DH + 1)],
                        es[:, hh, :],
                        start=(jt == 0), stop=(jt == NJT - 1),
                    )

            def make_div(h, ib, pavc, den_row):
                i0 = ib * IBLK

                def go():
                    recb_ps = ps.tile([P, IBLK], F32, tag="po", bufs=2,
                                      name=f"recb_{h}_{ib}")
                    nc.tensor.matmul(
                        recb_ps[:DH, :], ones_t[0:1, :], den_row[:],
                        start=True, stop=True,
                    )
                    recb_sb = stage_pool.tile([DH, IBLK], F32, tag="recb",
                                              bufs=2)
                    nc.vector.reciprocal_approx_fast(recb_sb[:], recb_ps[:DH, :])
                    e_c, e_lo = divmod(h * DH, P)
                    nc.vector.tensor_tensor(
                        outT[e_lo:e_lo + DH, e_c, i0:i0 + IBLK],
                        pavc[:], recb_sb[:], mybir.AluOpType.mult,
                    )
                return go

            def finalize_pass(hp, ib, pavs):
                for hh in range(2):
                    h = hp * 2 + hh
                    pavc = stage_pool.tile([DH, IBLK], F32, tag="pavc",
                                           bufs=2, name=f"pavc_{h}_{ib}")
                    nc.vector.tensor_copy(pavc[:], pavs[hh][:DH, :])
                    den_row = stage_pool.tile([1, IBLK], F32R, tag="den_row",
                                              bufs=2, name=f"den_{h}_{ib}")
                    nc.vector.tensor_copy(
                        den_row[:], pavs[hh][DH:DH + 1, :].bitcast(F32R))
                    div2.append(make_div(h, ib, pavc, den_row))

            def oproj_units(ib, tail=False):
                units = []
                for s in range(ib * (IBLK // P), (ib + 1) * (IBLK // P)):
                    def go(s=s, tail=tail):
                        oc = stage_pool.tile([P, DIM], F32, tag="oc", bufs=2,
                                             name=f"oc_{s}")
                        for nb in range(DIM // IBLK):
                            po = ps.tile([P, IBLK], F32, tag="po", bufs=2,
                                         name=f"po_{s}_{nb}")
                            for e in range(NEMBC):
                                nc.tensor.matmul(
                                    po[:],
                                    outT[:, e, s * P:(s + 1) * P],
                                    wo_sb[:, e, nb * IBLK:(nb + 1) * IBLK],
                                    start=(e == 0), stop=(e == NEMBC - 1),
                                )
                            d = oc[:, nb * IBLK:(nb + 1) * IBLK]
                            if tail:
                                nc.scalar.copy(d, po[:])
                            else:
                                nc.vector.tensor_copy(d, po[:])
                        nc.sync.dma_start(out[s * P:(s + 1) * P, :], oc[:])
                    units.append(go)
                return units

            # ---- prologue: chunk 0 + first projections ----
            emit_xform(0, ('vector', 'vector'))
            emit_proj(KT, "wk", 0, 0, 256, 'act')
            emit_proj(KT, "wk", 0, 256, 256, 'act')
            emit_proj(QT, "wq", 0, 0, IBLK, 'act')
            # remaining x DMAs + wv/wo (transfers overlap compute)
            emit_xdma(2)
            emit_xdma(3)
            emit_wdma("wv", wv)
            for j in range(4, 8):
                emit_xdma(j)
            wof = stage_pool.tile([P, NEMBC, DIM], F32, tag="wostage", bufs=1,
                                  name="wf_wo")
            nc.sync.dma_start(wof[:], wo.rearrange("(c p) n -> p c n", p=P))
            wo_sb = w_pool.tile([P, NEMBC, DIM], BF16, name="wb_wo")
            nc.gpsimd.tensor_copy(wo_sb[:], wof[:])

            # pair-1 projections, consumed as fillers at jt % 4 == 2
            fill = []
            for k in range(4):
                fill.append(lambda k=k: emit_proj(KT, "wk", 1, k * IBLK, IBLK,
                                                  'vector'))
            for k in range(4):
                fill.append(lambda k=k: emit_proj(QT, "wq", 1, k * IBLK, IBLK,
                                                  'vector'))

            div2 = []
            pending = []
            slots = [(hp, ib, jt)
                     for hp in range(2)
                     for ib in range(NIBLK)
                     for jt in range(NJT)]
            prev = None
            cur_pavs = None
            for hp, ib, jt in slots:
                first = (hp == 0 and ib == 0)
                if jt == 0:
                    cur_pavs = [
                        ps.tile([P, IBLK], F32, tag="pav", bufs=2,
                                name=f"pav_{hp}_{hh}_{ib}")
                        for hh in range(2)
                    ]
                if first and jt in (4, 8, 12):
                    k = jt // 4
                    emit_xform(k, ('vector', 'gpsimd'))
                    emit_proj(KT, "wk", 0, k * IBLK, IBLK, 'vector')
                es = emit_spair(ib, jt, hp)
                if prev is not None:
                    pes, ppavs, php, pib, pjt = prev
                    emit_av(ppavs, pes, pjt, php)
                    if pjt == NJT - 1:
                        finalize_pass(php, pib, ppavs)
                        if php == 1:
                            pending.extend(oproj_units(pib))
                if div2 and jt < 2:
                    div2.pop(0)()
                if first:
                    emit_vp(jt)
                    if jt == 10:
                        emit_proj(QT, "wq", 0, IBLK, IBLK, 'vector')
                else:
                    if hp == 0 and jt == 10 and ib < NIBLK - 1:
                        emit_proj(QT, "wq", 0, (ib + 1) * IBLK, IBLK, 'vector')
                    if fill and jt % 4 == 2:
                        fill.pop(0)()
                    if pending and jt % 2 == 1:
                        pending.pop(0)()
                prev = (es, cur_pavs, hp, ib, jt)

            # drain
            pes, ppavs, php, pib, pjt = prev
            emit_av(ppavs, pes, pjt, php)
            finalize_pass(php, pib, ppavs)
            for go in div2:
                go()
            for go in pending:
                go()
            for go in oproj_units(NIBLK - 1, tail=True):
                go()

    nc.compile()
    return nc


def shard_inputs(inputs):
    """Full inputs dict -> list of 8 per-core input dicts."""
    x = np.ascontiguousarray(inputs["x"], dtype=np.float32)
    maps = []
    for core in range(8):
        bi, hg = divmod(core, 4)
        sl = slice(hg * EMB_C, (hg + 1) * EMB_C)
        maps.append({
            "x": np.ascontiguousarray(x[bi]),
            "wq": np.ascontiguousarray(inputs["wq"][:, sl], np.float32),
            "wk": np.ascontiguousarray(inputs["wk"][:, sl], np.float32),
            "wv": np.ascontiguousarray(inputs["wv"][:, sl], np.float32),
            "bq": np.ascontiguousarray(inputs["bq"][sl], np.float32),
            "wo": np.ascontiguousarray(inputs["wo"][sl, :], np.float32),
            "ones64": np.ones((P, DH), np.float32),
        })
    return maps


def gather_outputs(results, inputs):
    out = np.zeros((2, SEQ, DIM), np.float32)
    for core in range(8):
        bi = core // 4
        out[bi] += results[core]["out"]
    bo = np.asarray(inputs["bo"], np.float32)
    bv = np.asarray(inputs["bv"], np.float32)
    wo = np.asarray(inputs["wo"], np.float32)
    out += bo + bv @ wo
    return out


_NC_CACHE = {}


def _get_nc(row_pack=True):
    if row_pack not in _NC_CACHE:
        _NC_CACHE[row_pack] = build_kernel(row_pack=row_pack)
    return _NC_CACHE[row_pack]


def run_sharded(inputs, trace=False, row_pack=True):
    """Returns (full_output [2,2048,1024] fp32, BassKernelResults)."""
    from concourse import bass_utils
    nc = _get_nc(row_pack)
    maps = shard_inputs(inputs)
    res = bass_utils.run_bass_kernel_spmd(
        nc, maps, core_ids=list(range(8)), trace=trace,
    )
    out = gather_outputs(res.results, inputs)
    return out, res


def kernel(**inputs):
    out, _ = run_sharded(inputs)
    return out
